# revision 17
# baseline (speedup 1.0000x reference)
"""FourierKAN adapter kernel for Trainium2 (8 NeuronCores, SPMD data-parallel).

out[t, d] = x[t, d] + c0[d] + sum_{k=1..3} a_k[d] sin(k x) + b_k[d] cos(k x)
x: [32768, 1024] f32, coeffs: [1024, 7] f32.

Math (phase form): a_k sin(kx) + b_k cos(kx) = r_k sin(k x + phi_k).
Per tile [128 tokens, 2048 cols] with w = f16(x / 2pi), PHI_k[d] = phi_k / 2pi:
    u_k = t - round(t),  t = k*w + PHI_k     (ONE fused custom DVE op per k:
                                              fp32 magic-constant rounding,
                                              |u_k| <= 0.5, f16 out)
    s   = Sin(2pi * u)                       (single batched ScalarE spline
                                              activation over all 3 harmonics,
                                              arg in [-pi, pi])
    m   = s * r                              (one batched DVE multiply)
PSUM accumulates 2pi*I @ w (the x term, f16) + ones-row @ c0 + I @ m_k
(identity matmuls, 512-col chunks); ScalarE evacuates PSUM -> f16 out;
host widens to f32. The custom DVE op FOURIER_RED_ANT is registered into
concourse.dve_ops at runtime (self-contained monkeypatch).

Sharding: x row-sharded across 8 cores; coeff-derived tables replicated.
Measured: ~194 us HW exec (vs 351 us baseline), rel err ~4.3e-4.
"""

import math
import os

import numpy as np

T = 32768
D = 1024
K = 3
N_CORES = 8
T_CORE = T // N_CORES  # 4096
P = 128
F = 2048               # megatile free dim (= 2 d-periods)
M32 = 12582912.0       # 1.5 * 2^23: fp32 round-to-nearest-int magic constant
TWO_PI = 2.0 * math.pi

LAST_RESULTS = None
_CACHED = {}


def _register_fred():
    """Register the fused range-reduction custom DVE op (idempotent).

    FOURIER_RED_ANT: out = t - ((t + C0) - C0) with t = Src0*C1 + Src1.
    C0 = M32 makes the inner add/sub a round-to-nearest-integer, so
    out = frac(t) in [-0.5, 0.5]. 5 ALU stages, fp32 internal, f16 out.
    """
    import concourse.dve_ops as dve_ops
    from concourse.dve_spec import C0, C1, Spec, Src0, Src1, lower, _has_src1
    from concourse.dve_uop import DveOpSpec

    name = "FOURIER_RED_ANT"
    for op in dve_ops.OPS:
        if op.name == name:
            return op

    t = Src0 * C1 + Src1
    n = (t + C0) - C0
    spec = Spec(
        body=t - n,
        reference=lambda in0, in1, s0, s1, imm2: (
            lambda tt: tt - (np.float32(tt + np.float32(s0)) - np.float32(s0))
        )(np.float32(in0) * np.float32(s1) + np.float32(in1)),
    )

    row = dve_ops._CUSTOM_DVE_ROW_BASE + len(dve_ops.OPS)
    assert row < 0x20, "custom DVE opcode rows exhausted"
    dve_ops._SUB_OPCODE_FOR_NAME[name] = row

    shas = {}
    for ver in ("v3", "v4"):
        s = DveOpSpec(
            name=name, opcode=row, uops=lower(spec, ver=ver),
            rd1_en=_has_src1(spec),
        )
        shas[ver] = s.sha(ver)
    op = dve_ops.DveOp(name, spec, False, shas)
    dve_ops.OPS.append(op)
    return op


def _build_nc(mode="full"):
    from concourse import bacc
    import concourse.mybir as mybir
    from concourse import tile

    f32 = mybir.dt.float32
    f16 = mybir.dt.float16
    Sin = mybir.ActivationFunctionType.Sin
    Copy = mybir.ActivationFunctionType.Copy

    fred = _register_fred()

    nc = bacc.Bacc("TRN2", target_bir_lowering=False, debug=False)

    x = nc.dram_tensor("x", [T_CORE, D], f32, kind="ExternalInput").ap()
    out = nc.dram_tensor("out", [T_CORE, D], f16, kind="ExternalOutput").ap()

    phi = {}
    for k in (1, 2, 3):
        phi[k] = nc.dram_tensor(f"phi{k}", [P, F], f16, kind="ExternalInput").ap()
    rall = nc.dram_tensor("rall", [P, 3 * F], f16, kind="ExternalInput").ap()
    c0row = nc.dram_tensor("c0row", [1, F], f16, kind="ExternalInput").ap()
    id16 = nc.dram_tensor("id16", [P, P], f16, kind="ExternalInput").ap()
    id2pi = nc.dram_tensor("id2pi", [P, P], f16, kind="ExternalInput").ap()
    ones1 = nc.dram_tensor("ones1", [1, P], f16, kind="ExternalInput").ap()

    xv = x.rearrange("(a b) d -> a (b d)", b=F // D)     # [2048, 2048]
    ov = out.rearrange("(a b) d -> a (b d)", b=F // D)
    n_tiles = xv.shape[0] // P  # 16

    with tile.TileContext(nc) as tc:
        with (
            tc.tile_pool(name="consts", bufs=1) as cpool,
            tc.tile_pool(name="xti", bufs=int(os.environ.get("KV2_IOBUFS", "4"))) as xtpool,
            tc.tile_pool(name="oto", bufs=int(os.environ.get("KV2_OBUFS", "4"))) as otpool,
            tc.tile_pool(name="work", bufs=int(os.environ.get("KV2_WBUFS", "7"))) as pool,
            tc.tile_pool(name="x16p", bufs=3) as xpool,
            tc.tile_pool(name="psum", bufs=2, space="PSUM") as ppool,
        ):
            phit = {}
            for k in (1, 2, 3):
                t_ = cpool.tile([P, F], f16, tag=f"phi{k}")
                nc.sync.dma_start(out=t_[:], in_=phi[k])
                phit[k] = t_
            rallt = cpool.tile([P, 3 * F], f16, tag="rall")
            nc.sync.dma_start(out=rallt[:], in_=rall)
            c0t = cpool.tile([1, F], f16, tag="c0row")
            nc.sync.dma_start(out=c0t[:], in_=c0row)
            id16t = cpool.tile([P, P], f16, tag="id16")
            nc.sync.dma_start(out=id16t[:], in_=id16)
            id2pit = cpool.tile([P, P], f16, tag="id2pi")
            nc.sync.dma_start(out=id2pit[:], in_=id2pi)
            ones1t = cpool.tile([1, P], f16, tag="ones1")
            nc.sync.dma_start(out=ones1t[:], in_=ones1)

            def body(i, c_lo, c_hi):
                rows = slice(i * P, (i + 1) * P)
                full = (c_hi - c_lo) == F
                xt = xtpool.tile([P, F], f32, tag="xt")
                nc.sync.dma_start(out=xt[:, c_lo:c_hi], in_=xv[rows, c_lo:c_hi])

                w = xpool.tile([P, F], f16, tag="w")
                nc.scalar.activation(w[:, c_lo:c_hi], xt[:, c_lo:c_hi], Copy,
                                     bias=0.0, scale=float(1.0 / TWO_PI))

                u = pool.tile([P, 3 * F], f16, tag="u")
                for k in (1, 2, 3):
                    nc.vector._custom_dve(
                        fred,
                        out=u[:, (k - 1) * F + c_lo:(k - 1) * F + c_hi],
                        in0=w[:, c_lo:c_hi], in1=phit[k][:, c_lo:c_hi],
                        s0=M32, s1=float(k),
                    )

                # Sin in-place on u, then amplitude multiply in-place:
                # m (== u tile) = sin(2*pi*u) * r
                s = u
                m = s
                if full:
                    nc.scalar.activation(s[:], u[:], Sin, bias=0.0,
                                         scale=float(TWO_PI))
                    nc.vector.tensor_mul(out=m[:], in0=s[:], in1=rallt[:])
                else:
                    for k in (1, 2, 3):
                        ks = slice((k - 1) * F + c_lo, (k - 1) * F + c_hi)
                        nc.scalar.activation(s[:, ks], u[:, ks], Sin,
                                             bias=0.0, scale=float(TWO_PI))
                        nc.vector.tensor_mul(out=m[:, ks], in0=s[:, ks],
                                             in1=rallt[:, ks])

                # w (x-term) and c0 matmuls don't depend on the sin chain:
                # emit them first so TensorE starts while DVE runs FRED.
                ps = ppool.tile([P, F], f32, tag="ps")
                for c in range(c_lo // 512, c_hi // 512):
                    sl = slice(c * 512, (c + 1) * 512)
                    nc.tensor.matmul(ps[:, sl], id2pit[:], w[:, sl],
                                     start=True, stop=False)
                    nc.tensor.matmul(ps[:, sl], ones1t[:], c0t[:, sl],
                                     start=False, stop=False)
                for c in range(c_lo // 512, c_hi // 512):
                    sl = slice(c * 512, (c + 1) * 512)
                    for ki in range(3):
                        slm = slice(ki * F + c * 512, ki * F + (c + 1) * 512)
                        nc.tensor.matmul(ps[:, sl], id16t[:], m[:, slm],
                                         start=False, stop=(ki == 2))

                ot = otpool.tile([P, F], f16, tag="ot")
                nc.scalar.activation(ot[:, c_lo:c_hi], ps[:, c_lo:c_hi], Copy,
                                     bias=0.0, scale=1.0)
                nc.sync.dma_start(out=ov[rows, c_lo:c_hi], in_=ot[:, c_lo:c_hi])

            for i in range(n_tiles):
                body(i, 0, F)

    nc.compile()
    return nc


def _host_tables(coeffs: np.ndarray) -> dict:
    c = coeffs.astype(np.float64)
    nrep = F // D
    tabs = {"c0row": np.tile(np.float16(c[:, 0]), nrep)[None, :]}
    rparts = []
    for k in (1, 2, 3):
        a = c[:, 2 * k - 1]
        b = c[:, 2 * k]
        r = np.hypot(a, b)
        phi = np.arctan2(b, a)
        tabs[f"phi{k}"] = np.tile(
            np.float16(phi / TWO_PI), (P, nrep))
        rparts.append(np.tile(np.float16(r), nrep))
    tabs["rall"] = np.tile(np.concatenate(rparts)[None, :], (P, 1))
    tabs["id16"] = np.eye(P, dtype=np.float16)
    tabs["id2pi"] = (np.eye(P) * 2 * np.pi).astype(np.float16)
    tabs["ones1"] = np.ones((1, P), dtype=np.float16)
    return tabs


def kernel(x: np.ndarray, coeffs: np.ndarray) -> np.ndarray:
    global LAST_RESULTS
    from concourse.bass_utils import run_bass_kernel_spmd

    x = np.ascontiguousarray(np.asarray(x, dtype=np.float32))
    coeffs = np.asarray(coeffs, dtype=np.float32)
    assert x.shape == (T, D) and coeffs.shape == (D, 2 * K + 1)

    mode = os.environ.get("KV2_MODE", "full")
    key = ("nc", mode)
    if key not in _CACHED:
        _CACHED[key] = _build_nc(mode)
    nc = _CACHED[key]

    tabs = _host_tables(coeffs)
    in_maps = []
    for i in range(N_CORES):
        m = {"x": x[i * T_CORE:(i + 1) * T_CORE]}
        m.update(tabs)
        in_maps.append(m)

    trace = bool(os.environ.get("BASS_TRACE"))
    try:
        res = run_bass_kernel_spmd(
            nc, in_maps, list(range(N_CORES)), trace=trace,
        )
    except ModuleNotFoundError:
        res = run_bass_kernel_spmd(
            nc, in_maps, list(range(N_CORES)), trace=False,
        )
    LAST_RESULTS = res
    out = np.concatenate([res.results[i]["out"] for i in range(N_CORES)], axis=0)
    return out.astype(np.float32)


# revision 18
# speedup vs baseline: 1.1677x; 1.1677x over previous
"""FourierKAN adapter kernel for Trainium2 (8 NeuronCores, SPMD data-parallel).

out[t, d] = x[t, d] + c0[d] + sum_{k=1..3} a_k[d] sin(k x) + b_k[d] cos(k x)
x: [32768, 1024] f32, coeffs: [1024, 7] f32.

Math (phase form): a_k sin(kx) + b_k cos(kx) = r_k sin(k x + phi_k).
Per tile [128 tokens, 2048 cols] with w = f16(x / 2pi), PHI_k[d] = phi_k / 2pi:
    u_k = t - round(t),  t = k*w + PHI_k     (ONE fused custom DVE op per k:
                                              fp32 magic-constant rounding,
                                              |u_k| <= 0.5, f16 out)
    s   = Sin(2pi * u)                       (single batched ScalarE spline
                                              activation over all 3 harmonics,
                                              arg in [-pi, pi])
    m   = s * r                              (one batched DVE multiply)
PSUM accumulates 2pi*I @ w (the x term, f16) + ones-row @ c0 + I @ m_k
(identity matmuls, 512-col chunks); ScalarE evacuates PSUM -> f16 out;
host widens to f32. The custom DVE op FOURIER_RED_ANT is registered into
concourse.dve_ops at runtime (self-contained monkeypatch).

Sharding: x row-sharded across 8 cores; coeff-derived tables replicated.
Measured: ~194 us HW exec (vs 351 us baseline), rel err ~4.3e-4.
"""

import math
import os

import numpy as np

T = 32768
D = 1024
K = 3
N_CORES = 8
T_CORE = T // N_CORES  # 4096
P = 128
F = 2048               # megatile free dim (= 2 d-periods)
M32 = 12582912.0       # 1.5 * 2^23: fp32 round-to-nearest-int magic constant
TWO_PI = 2.0 * math.pi

LAST_RESULTS = None
_CACHED = {}


def _register_fred():
    """Register the fused range-reduction custom DVE op (idempotent).

    FOURIER_RED_ANT: out = t - ((t + C0) - C0) with t = Src0*C1 + Src1.
    C0 = M32 makes the inner add/sub a round-to-nearest-integer, so
    out = frac(t) in [-0.5, 0.5]. 5 ALU stages, fp32 internal, f16 out.
    """
    import concourse.dve_ops as dve_ops
    from concourse.dve_spec import C0, C1, Spec, Src0, Src1, lower, _has_src1
    from concourse.dve_uop import DveOpSpec

    name = "FOURIER_RED_ANT"
    for op in dve_ops.OPS:
        if op.name == name:
            return op

    t = Src0 * C1 + Src1
    n = (t + C0) - C0
    spec = Spec(
        body=t - n,
        reference=lambda in0, in1, s0, s1, imm2: (
            lambda tt: tt - (np.float32(tt + np.float32(s0)) - np.float32(s0))
        )(np.float32(in0) * np.float32(s1) + np.float32(in1)),
    )

    row = dve_ops._CUSTOM_DVE_ROW_BASE + len(dve_ops.OPS)
    assert row < 0x20, "custom DVE opcode rows exhausted"
    dve_ops._SUB_OPCODE_FOR_NAME[name] = row

    shas = {}
    for ver in ("v3", "v4"):
        s = DveOpSpec(
            name=name, opcode=row, uops=lower(spec, ver=ver),
            rd1_en=_has_src1(spec),
        )
        shas[ver] = s.sha(ver)
    op = dve_ops.DveOp(name, spec, False, shas)
    dve_ops.OPS.append(op)
    return op


def _build_nc(mode="full"):
    from concourse import bacc
    import concourse.mybir as mybir
    from concourse import tile

    f32 = mybir.dt.float32
    f16 = mybir.dt.float16
    Sin = mybir.ActivationFunctionType.Sin
    Copy = mybir.ActivationFunctionType.Copy

    fred = _register_fred()

    nc = bacc.Bacc("TRN2", target_bir_lowering=False, debug=False)

    x = nc.dram_tensor("x", [T_CORE, D], f32, kind="ExternalInput").ap()
    out = nc.dram_tensor("out", [T_CORE, D], f16, kind="ExternalOutput").ap()

    phi = {}
    for k in (1, 2, 3):
        phi[k] = nc.dram_tensor(f"phi{k}", [P, F], f16, kind="ExternalInput").ap()
    rall = nc.dram_tensor("rall", [P, 3 * F], f16, kind="ExternalInput").ap()
    c0row = nc.dram_tensor("c0row", [1, F], f16, kind="ExternalInput").ap()
    id16 = nc.dram_tensor("id16", [P, P], f16, kind="ExternalInput").ap()
    id2pi = nc.dram_tensor("id2pi", [P, P], f16, kind="ExternalInput").ap()
    ones1 = nc.dram_tensor("ones1", [1, P], f16, kind="ExternalInput").ap()

    xv = x.rearrange("(a b) d -> a (b d)", b=F // D)     # [2048, 2048]
    ov = out.rearrange("(a b) d -> a (b d)", b=F // D)
    n_tiles = xv.shape[0] // P  # 16

    with tile.TileContext(nc) as tc:
        with (
            tc.tile_pool(name="consts", bufs=1) as cpool,
            tc.tile_pool(name="xti", bufs=int(os.environ.get("KV2_IOBUFS", "4"))) as xtpool,
            tc.tile_pool(name="oto", bufs=int(os.environ.get("KV2_OBUFS", "4"))) as otpool,
            tc.tile_pool(name="work", bufs=int(os.environ.get("KV2_WBUFS", "7"))) as pool,
            tc.tile_pool(name="x16p", bufs=3) as xpool,
            tc.tile_pool(name="psum", bufs=2, space="PSUM") as ppool,
        ):
            phit = {}
            for k in (1, 2, 3):
                t_ = cpool.tile([P, F], f16, tag=f"phi{k}")
                nc.sync.dma_start(out=t_[:], in_=phi[k])
                phit[k] = t_
            rallt = cpool.tile([P, 3 * F], f16, tag="rall")
            nc.sync.dma_start(out=rallt[:], in_=rall)
            c0t = cpool.tile([1, F], f16, tag="c0row")
            nc.sync.dma_start(out=c0t[:], in_=c0row)
            id16t = cpool.tile([P, P], f16, tag="id16")
            nc.sync.dma_start(out=id16t[:], in_=id16)
            id2pit = cpool.tile([P, P], f16, tag="id2pi")
            nc.sync.dma_start(out=id2pit[:], in_=id2pi)
            ones1t = cpool.tile([1, P], f16, tag="ones1")
            nc.sync.dma_start(out=ones1t[:], in_=ones1)

            def body(i, c_lo, c_hi):
                rows = slice(i * P, (i + 1) * P)
                full = (c_hi - c_lo) == F
                xt = xtpool.tile([P, F], f32, tag="xt")
                nc.sync.dma_start(out=xt[:, c_lo:c_hi], in_=xv[rows, c_lo:c_hi])

                w = xpool.tile([P, F], f16, tag="w")
                nc.scalar.activation(w[:, c_lo:c_hi], xt[:, c_lo:c_hi], Copy,
                                     bias=0.0, scale=float(1.0 / TWO_PI))

                u = pool.tile([P, 3 * F], f16, tag="u")
                for k in (1, 2, 3):
                    nc.vector._custom_dve(
                        fred,
                        out=u[:, (k - 1) * F + c_lo:(k - 1) * F + c_hi],
                        in0=w[:, c_lo:c_hi], in1=phit[k][:, c_lo:c_hi],
                        s0=M32, s1=float(k),
                    )

                # Sin in-place on u, then amplitude multiply in-place:
                # m (== u tile) = sin(2*pi*u) * r
                s = u
                m = s
                if full:
                    nc.scalar.activation(s[:], u[:], Sin, bias=0.0,
                                         scale=float(TWO_PI))
                    nc.vector.tensor_mul(out=m[:], in0=s[:], in1=rallt[:])
                else:
                    for k in (1, 2, 3):
                        ks = slice((k - 1) * F + c_lo, (k - 1) * F + c_hi)
                        nc.scalar.activation(s[:, ks], u[:, ks], Sin,
                                             bias=0.0, scale=float(TWO_PI))
                        nc.vector.tensor_mul(out=m[:, ks], in0=s[:, ks],
                                             in1=rallt[:, ks])

                # w (x-term) and c0 matmuls don't depend on the sin chain:
                # emit them first so TensorE starts while DVE runs FRED.
                ps = ppool.tile([P, F], f32, tag="ps")
                for c in range(c_lo // 512, c_hi // 512):
                    sl = slice(c * 512, (c + 1) * 512)
                    nc.tensor.matmul(ps[:, sl], id2pit[:], w[:, sl],
                                     start=True, stop=False)
                    nc.tensor.matmul(ps[:, sl], ones1t[:], c0t[:, sl],
                                     start=False, stop=False)
                for c in range(c_lo // 512, c_hi // 512):
                    sl = slice(c * 512, (c + 1) * 512)
                    for ki in range(3):
                        slm = slice(ki * F + c * 512, ki * F + (c + 1) * 512)
                        nc.tensor.matmul(ps[:, sl], id16t[:], m[:, slm],
                                         start=False, stop=(ki == 2))

                ot = otpool.tile([P, F], f16, tag="ot")
                nc.scalar.activation(ot[:, c_lo:c_hi], ps[:, c_lo:c_hi], Copy,
                                     bias=0.0, scale=1.0)
                nc.sync.dma_start(out=ov[rows, c_lo:c_hi], in_=ot[:, c_lo:c_hi])

            # Halve the first and last tiles: shortens pipeline fill/drain
            # and (measured) yields a better steady-state schedule.
            schedule = [(0, 0, F // 2), (0, F // 2, F)]
            schedule += [(i, 0, F) for i in range(1, n_tiles - 1)]
            schedule += [(n_tiles - 1, 0, F // 2), (n_tiles - 1, F // 2, F)]
            for i, c_lo, c_hi in schedule:
                body(i, c_lo, c_hi)

    nc.compile()
    return nc


def _host_tables(coeffs: np.ndarray) -> dict:
    c = coeffs.astype(np.float64)
    nrep = F // D
    tabs = {"c0row": np.tile(np.float16(c[:, 0]), nrep)[None, :]}
    rparts = []
    for k in (1, 2, 3):
        a = c[:, 2 * k - 1]
        b = c[:, 2 * k]
        r = np.hypot(a, b)
        phi = np.arctan2(b, a)
        tabs[f"phi{k}"] = np.tile(
            np.float16(phi / TWO_PI), (P, nrep))
        rparts.append(np.tile(np.float16(r), nrep))
    tabs["rall"] = np.tile(np.concatenate(rparts)[None, :], (P, 1))
    tabs["id16"] = np.eye(P, dtype=np.float16)
    tabs["id2pi"] = (np.eye(P) * 2 * np.pi).astype(np.float16)
    tabs["ones1"] = np.ones((1, P), dtype=np.float16)
    return tabs


def kernel(x: np.ndarray, coeffs: np.ndarray) -> np.ndarray:
    global LAST_RESULTS
    from concourse.bass_utils import run_bass_kernel_spmd

    x = np.ascontiguousarray(np.asarray(x, dtype=np.float32))
    coeffs = np.asarray(coeffs, dtype=np.float32)
    assert x.shape == (T, D) and coeffs.shape == (D, 2 * K + 1)

    mode = os.environ.get("KV2_MODE", "full")
    key = ("nc", mode)
    if key not in _CACHED:
        _CACHED[key] = _build_nc(mode)
    nc = _CACHED[key]

    tabs = _host_tables(coeffs)
    in_maps = []
    for i in range(N_CORES):
        m = {"x": x[i * T_CORE:(i + 1) * T_CORE]}
        m.update(tabs)
        in_maps.append(m)

    trace = bool(os.environ.get("BASS_TRACE"))
    try:
        res = run_bass_kernel_spmd(
            nc, in_maps, list(range(N_CORES)), trace=trace,
        )
    except ModuleNotFoundError:
        res = run_bass_kernel_spmd(
            nc, in_maps, list(range(N_CORES)), trace=False,
        )
    LAST_RESULTS = res
    out = np.concatenate([res.results[i]["out"] for i in range(N_CORES)], axis=0)
    return out.astype(np.float32)


# revision 20
# speedup vs baseline: 1.1870x; 1.0165x over previous
"""FourierKAN adapter kernel for Trainium2 (8 NeuronCores, SPMD data-parallel).

out[t, d] = x[t, d] + c0[d] + sum_{k=1..3} a_k[d] sin(k x) + b_k[d] cos(k x)
x: [32768, 1024] f32, coeffs: [1024, 7] f32.

Math (phase form): a_k sin(kx) + b_k cos(kx) = r_k sin(k x + phi_k).
Per tile [128 tokens, 2048 cols] with w = f16(x / 2pi), PHI_k[d] = phi_k / 2pi:
    u_k = t - round(t),  t = k*w + PHI_k     (ONE fused custom DVE op per k:
                                              fp32 magic-constant rounding,
                                              |u_k| <= 0.5, f16 out)
    s   = Sin(2pi * u)                       (single batched ScalarE spline
                                              activation over all 3 harmonics,
                                              arg in [-pi, pi])
    m   = s * r                              (one batched DVE multiply)
PSUM accumulates 2pi*I @ w (the x term, f16) + ones-row @ c0 + I @ m_k
(identity matmuls, 512-col chunks); ScalarE evacuates PSUM -> f16 out;
host widens to f32. The custom DVE op FOURIER_RED_ANT is registered into
concourse.dve_ops at runtime (self-contained monkeypatch).

Sharding: x row-sharded across 8 cores; coeff-derived tables replicated.
Measured: ~196 us HW exec (vs 351 us baseline), rel err ~4.3e-4.
"""

import math
import os

import numpy as np

T = 32768
D = 1024
K = 3
N_CORES = 8
T_CORE = T // N_CORES  # 4096
P = 128
F = 2048               # megatile free dim (= 2 d-periods)
M32 = 12582912.0       # 1.5 * 2^23: fp32 round-to-nearest-int magic constant
TWO_PI = 2.0 * math.pi

LAST_RESULTS = None
_CACHED = {}


def _register_fred():
    """Register the fused range-reduction custom DVE op (idempotent).

    FOURIER_RED_ANT: out = t - ((t + C0) - C0) with t = Src0*C1 + Src1.
    C0 = M32 makes the inner add/sub a round-to-nearest-integer, so
    out = frac(t) in [-0.5, 0.5]. 5 ALU stages, fp32 internal, f16 out.
    """
    import concourse.dve_ops as dve_ops
    from concourse.dve_spec import C0, C1, Spec, Src0, Src1, lower, _has_src1
    from concourse.dve_uop import DveOpSpec

    name = "FOURIER_RED_ANT"
    for op in dve_ops.OPS:
        if op.name == name:
            return op

    t = Src0 * C1 + Src1
    n = (t + C0) - C0
    spec = Spec(
        body=t - n,
        reference=lambda in0, in1, s0, s1, imm2: (
            lambda tt: tt - (np.float32(tt + np.float32(s0)) - np.float32(s0))
        )(np.float32(in0) * np.float32(s1) + np.float32(in1)),
    )

    row = dve_ops._CUSTOM_DVE_ROW_BASE + len(dve_ops.OPS)
    assert row < 0x20, "custom DVE opcode rows exhausted"
    dve_ops._SUB_OPCODE_FOR_NAME[name] = row

    shas = {}
    for ver in ("v3", "v4"):
        s = DveOpSpec(
            name=name, opcode=row, uops=lower(spec, ver=ver),
            rd1_en=_has_src1(spec),
        )
        shas[ver] = s.sha(ver)
    op = dve_ops.DveOp(name, spec, False, shas)
    dve_ops.OPS.append(op)
    return op


def _build_nc(mode="full"):
    from concourse import bacc
    import concourse.mybir as mybir
    from concourse import tile

    f32 = mybir.dt.float32
    f16 = mybir.dt.float16
    Sin = mybir.ActivationFunctionType.Sin
    Copy = mybir.ActivationFunctionType.Copy

    fred = _register_fred()

    nc = bacc.Bacc("TRN2", target_bir_lowering=False, debug=False)

    x = nc.dram_tensor("x", [T_CORE, D], f32, kind="ExternalInput").ap()
    out = nc.dram_tensor("out", [T_CORE, D], f16, kind="ExternalOutput").ap()

    phi = {}
    for k in (1, 2, 3):
        phi[k] = nc.dram_tensor(f"phi{k}", [P, F], f16, kind="ExternalInput").ap()
    rall = nc.dram_tensor("rall", [P, 3 * F], f16, kind="ExternalInput").ap()
    c0row = nc.dram_tensor("c0row", [1, F], f16, kind="ExternalInput").ap()
    id16 = nc.dram_tensor("id16", [P, P], f16, kind="ExternalInput").ap()
    id2pi = nc.dram_tensor("id2pi", [P, P], f16, kind="ExternalInput").ap()
    ones1 = nc.dram_tensor("ones1", [1, P], f16, kind="ExternalInput").ap()

    xv = x.rearrange("(a b) d -> a (b d)", b=F // D)     # [2048, 2048]
    ov = out.rearrange("(a b) d -> a (b d)", b=F // D)
    n_tiles = xv.shape[0] // P  # 16

    with tile.TileContext(nc) as tc:
        with (
            tc.tile_pool(name="consts", bufs=1) as cpool,
            tc.tile_pool(name="xti", bufs=int(os.environ.get("KV2_IOBUFS", "4"))) as xtpool,
            tc.tile_pool(name="oto", bufs=int(os.environ.get("KV2_OBUFS", "4"))) as otpool,
            tc.tile_pool(name="work", bufs=int(os.environ.get("KV2_WBUFS", "7"))) as pool,
            tc.tile_pool(name="x16p", bufs=3) as xpool,
            tc.tile_pool(name="psum", bufs=2, space="PSUM") as ppool,
        ):
            phit = {}
            for k in (1, 2, 3):
                t_ = cpool.tile([P, F], f16, tag=f"phi{k}")
                nc.sync.dma_start(out=t_[:], in_=phi[k])
                phit[k] = t_
            rallt = cpool.tile([P, 3 * F], f16, tag="rall")
            nc.sync.dma_start(out=rallt[:], in_=rall)
            c0t = cpool.tile([1, F], f16, tag="c0row")
            nc.sync.dma_start(out=c0t[:], in_=c0row)
            id16t = cpool.tile([P, P], f16, tag="id16")
            nc.sync.dma_start(out=id16t[:], in_=id16)
            id2pit = cpool.tile([P, P], f16, tag="id2pi")
            nc.sync.dma_start(out=id2pit[:], in_=id2pi)
            ones1t = cpool.tile([1, P], f16, tag="ones1")
            nc.sync.dma_start(out=ones1t[:], in_=ones1)

            # 3-stage software pipeline. Emitting A(i+2); B(i+1); C(i)
            # keeps every engine's issue queue stocked with ready work:
            # Scalar never blocks a w-copy behind an evac that is still
            # waiting on TensorE, and Vector never blocks next-tile FREDs
            # behind a multiply waiting on Sin.
            def stage_a(i, c_lo, c_hi):
                rows = slice(i * P, (i + 1) * P)
                xt = xtpool.tile([P, F], f32, tag="xt")
                nc.sync.dma_start(out=xt[:, c_lo:c_hi], in_=xv[rows, c_lo:c_hi])

                w = xpool.tile([P, F], f16, tag="w")
                nc.scalar.activation(w[:, c_lo:c_hi], xt[:, c_lo:c_hi], Copy,
                                     bias=0.0, scale=float(1.0 / TWO_PI))

                u = pool.tile([P, 3 * F], f16, tag="u")
                for k in (1, 2, 3):
                    nc.vector._custom_dve(
                        fred,
                        out=u[:, (k - 1) * F + c_lo:(k - 1) * F + c_hi],
                        in0=w[:, c_lo:c_hi], in1=phit[k][:, c_lo:c_hi],
                        s0=M32, s1=float(k),
                    )
                return (i, c_lo, c_hi, w, u)

            def stage_b(st):
                i, c_lo, c_hi, w, u = st
                full = (c_hi - c_lo) == F
                # Sin in-place on u, then amplitude multiply in-place:
                # m (== u tile) = sin(2*pi*u) * r
                s = u
                m = s
                if full:
                    nc.scalar.activation(s[:], u[:], Sin, bias=0.0,
                                         scale=float(TWO_PI))
                    nc.vector.tensor_mul(out=m[:], in0=s[:], in1=rallt[:])
                else:
                    for k in (1, 2, 3):
                        ks = slice((k - 1) * F + c_lo, (k - 1) * F + c_hi)
                        nc.scalar.activation(s[:, ks], u[:, ks], Sin,
                                             bias=0.0, scale=float(TWO_PI))
                        nc.vector.tensor_mul(out=m[:, ks], in0=s[:, ks],
                                             in1=rallt[:, ks])

                ps = ppool.tile([P, F], f32, tag="ps")
                for c in range(c_lo // 512, c_hi // 512):
                    sl = slice(c * 512, (c + 1) * 512)
                    nc.tensor.matmul(ps[:, sl], id2pit[:], w[:, sl],
                                     start=True, stop=False)
                    nc.tensor.matmul(ps[:, sl], ones1t[:], c0t[:, sl],
                                     start=False, stop=False)
                for c in range(c_lo // 512, c_hi // 512):
                    sl = slice(c * 512, (c + 1) * 512)
                    for ki in range(3):
                        slm = slice(ki * F + c * 512, ki * F + (c + 1) * 512)
                        nc.tensor.matmul(ps[:, sl], id16t[:], m[:, slm],
                                         start=False, stop=(ki == 2))
                return (i, c_lo, c_hi, ps)

            def stage_c(st):
                i, c_lo, c_hi, ps = st
                rows = slice(i * P, (i + 1) * P)
                ot = otpool.tile([P, F], f16, tag="ot")
                nc.scalar.activation(ot[:, c_lo:c_hi], ps[:, c_lo:c_hi], Copy,
                                     bias=0.0, scale=1.0)
                nc.sync.dma_start(out=ov[rows, c_lo:c_hi], in_=ot[:, c_lo:c_hi])

            # Halve the first and last tiles: shortens pipeline fill/drain.
            schedule = [(0, 0, F // 2), (0, F // 2, F)]
            schedule += [(i, 0, F) for i in range(1, n_tiles - 1)]
            schedule += [(n_tiles - 1, 0, F // 2), (n_tiles - 1, F // 2, F)]

            a_st = [stage_a(*schedule[0])]
            b_st = []
            for j in range(1, len(schedule) + 2):
                if j < len(schedule):
                    a_st.append(stage_a(*schedule[j]))
                if a_st and j >= 1:
                    b_st.append(stage_b(a_st.pop(0)))
                if b_st and j >= 2:
                    stage_c(b_st.pop(0))

    nc.compile()
    return nc


def _host_tables(coeffs: np.ndarray) -> dict:
    c = coeffs.astype(np.float64)
    nrep = F // D
    tabs = {"c0row": np.tile(np.float16(c[:, 0]), nrep)[None, :]}
    rparts = []
    for k in (1, 2, 3):
        a = c[:, 2 * k - 1]
        b = c[:, 2 * k]
        r = np.hypot(a, b)
        phi = np.arctan2(b, a)
        tabs[f"phi{k}"] = np.tile(
            np.float16(phi / TWO_PI), (P, nrep))
        rparts.append(np.tile(np.float16(r), nrep))
    tabs["rall"] = np.tile(np.concatenate(rparts)[None, :], (P, 1))
    tabs["id16"] = np.eye(P, dtype=np.float16)
    tabs["id2pi"] = (np.eye(P) * 2 * np.pi).astype(np.float16)
    tabs["ones1"] = np.ones((1, P), dtype=np.float16)
    return tabs


def kernel(x: np.ndarray, coeffs: np.ndarray) -> np.ndarray:
    global LAST_RESULTS
    from concourse.bass_utils import run_bass_kernel_spmd

    x = np.ascontiguousarray(np.asarray(x, dtype=np.float32))
    coeffs = np.asarray(coeffs, dtype=np.float32)
    assert x.shape == (T, D) and coeffs.shape == (D, 2 * K + 1)

    mode = os.environ.get("KV2_MODE", "full")
    key = ("nc", mode)
    if key not in _CACHED:
        _CACHED[key] = _build_nc(mode)
    nc = _CACHED[key]

    tabs = _host_tables(coeffs)
    in_maps = []
    for i in range(N_CORES):
        m = {"x": x[i * T_CORE:(i + 1) * T_CORE]}
        m.update(tabs)
        in_maps.append(m)

    trace = bool(os.environ.get("BASS_TRACE"))
    try:
        res = run_bass_kernel_spmd(
            nc, in_maps, list(range(N_CORES)), trace=trace,
        )
    except ModuleNotFoundError:
        res = run_bass_kernel_spmd(
            nc, in_maps, list(range(N_CORES)), trace=False,
        )
    LAST_RESULTS = res
    out = np.concatenate([res.results[i]["out"] for i in range(N_CORES)], axis=0)
    return out.astype(np.float32)
